# revision 1
# baseline (speedup 1.0000x reference)
"""Trainium2 Bass kernel v2 for nn_Engram (hashed n-gram embedding + ShortConv gate).

Self-contained: hardcodes all shapes. kernel(**inputs) -> full output [4,2048,4,1024].

Sharding: 8 cores; core c handles batch b=c//2, token half h=c%2 (1024 tokens).
Embedding table replicated per core, quad-packed (4 rows / 512B unit) per head so
int16 dma_gather indices cover a head's vocab.

v2 changes vs baseline:
- token-half (hf) pipeline: select/conv (DVE/Pool) of half 1 overlaps key/value
  matmuls (PE) of half 0; PSUM pools hoisted so halves don't serialize.
- full-width conv with in-place RMSNorm (xn overwrites xb); taps split DVE/Pool.
- select: one big mask-mult (in-place on gathered tile) + two contiguous adds
  (Pool) instead of strided tensor_reduce.
- gathers spread over 4 SWDGE queues.
- bf16 output store (halves store traffic).
"""
import sys
sys.path.insert(0, "/opt/trn_rl_repo")
from contextlib import ExitStack

import numpy as np
import ml_dtypes

import concourse.bass as bass
import concourse.mybir as mybir
import concourse.tile as tile
from concourse import bacc
from concourse.bass_utils import run_bass_kernel_spmd

BF16 = ml_dtypes.bfloat16
F32 = mybir.dt.float32
BF = mybir.dt.bfloat16
I16 = mybir.dt.int16
MULT = mybir.AluOpType.mult
ADD = mybir.AluOpType.add
AF = mybir.ActivationFunctionType

B, T, G, HID = 4, 2048, 4, 1024
NH, D = 16, 64
C = G * HID
KS, DIL = 4, 3
HALO = (KS - 1) * DIL  # 9
W = 1024 + HALO        # 1033
EPS_SC, EPS_RMS = 1e-5, 1e-6
VOCAB_PER_NGRAM = (129280, 129280)
N_CORES = 8
TOK = (B * T) // N_CORES  # 1024
NCH = 32
HT = 512               # tokens per hf half
NQ = 4                 # swdge queues
import os as _os
SIM_SILU = _os.environ.get("SIM_SILU", "0") == "1"  # CoreSim lacks Silu; decompose when simulating


def _isprime(n):
    if n < 2:
        return False
    if n % 2 == 0:
        return n == 2
    i = 3
    while i * i <= n:
        if n % i == 0:
            return False
        i += 2
    return True


def _head_vocab_sizes():
    seen, out = set(), []
    for v in VOCAB_PER_NGRAM:
        start = v - 1
        for _ in range(NH // 2):
            c = start + 1
            while (not _isprime(c)) or (c in seen):
                c += 1
            seen.add(c)
            out.append(c)
            start = c
    return out


LIST_OF_N = _head_vocab_sizes()
OFFSETS = np.concatenate([[0], np.cumsum(LIST_OF_N[:-1])]).astype(np.int64)
NUNITS = int(max((n + 3) // 4 for n in LIST_OF_N))

_CACHE = {}

# psum free-dim chunking (2048B per bank -> max 512 f32 columns)
CHUNKS = [(0, 259), (259, 259), (518, 259), (777, 256)]


def _build_bass(reps=1, hw_loop=False, phases=(1, 2, 3)):
    nc = bacc.Bacc("TRN2", target_bir_lowering=False, num_swdge_queues=NQ)

    hT_d = nc.dram_tensor("hT", [128, NCH, W], BF, kind="ExternalInput")
    qtab_d = nc.dram_tensor("qtab", [NH, NUNITS, 256], BF, kind="ExternalInput")
    idx_d = nc.dram_tensor("idx16", [128, 2, NH, HT // 16], I16, kind="ExternalInput")
    mask_d = nc.dram_tensor("mask", [128, 2, 4, 16, 4], BF, kind="ExternalInput")
    vwT_d = nc.dram_tensor("vwT", [8, 128, 8, 128], BF, kind="ExternalInput")   # [dt, e_in, e_hi, d_in]
    kwT_d = nc.dram_tensor("kwT", [G, 8, 128, 8, 128], BF, kind="ExternalInput")  # [g, ot, e_in, e_hi, o_in]
    vtap_d = nc.dram_tensor("vtap", [128, NCH, KS], BF, kind="ExternalInput")
    diag_d = nc.dram_tensor("diag", [G, 8, KS, 128, 128], BF, kind="ExternalInput")
    w12_d = nc.dram_tensor("w12pad", [128, G, 8, 4], BF, kind="ExternalInput")
    ones_d = nc.dram_tensor("onespad", [128, G, 4], BF, kind="ExternalInput")
    oh_d = nc.dram_tensor("onehot", [G, 4, 128], BF, kind="ExternalInput")
    ident_d = nc.dram_tensor("ident", [128, 128], BF, kind="ExternalInput")
    kb_d = nc.dram_tensor("kb", [128, G, 8], F32, kind="ExternalInput")
    vb_d = nc.dram_tensor("vb", [128, 8], F32, kind="ExternalInput")
    outT_d = nc.dram_tensor("outT", [G, 8, 128, TOK], BF, kind="ExternalOutput")

    with ExitStack() as ctx:
        tc = ctx.enter_context(tile.TileContext(nc))
        const = ctx.enter_context(tc.tile_pool(name="const", bufs=1))
        p_xn = ctx.enter_context(tc.tile_pool(name="xn", bufs=1))
        p_x2 = ctx.enter_context(tc.tile_pool(name="x2", bufs=1))
        p_rbc = ctx.enter_context(tc.tile_pool(name="rbc", bufs=2))
        p_row = ctx.enter_context(tc.tile_pool(name="row", bufs=3))
        p_emb = ctx.enter_context(tc.tile_pool(name="emb", bufs=1))
        p_ga = ctx.enter_context(tc.tile_pool(name="ga", bufs=2))
        p_eba = ctx.enter_context(tc.tile_pool(name="eba", bufs=1))
        p_cv = ctx.enter_context(tc.tile_pool(name="cv", bufs=2))
        p_tap = ctx.enter_context(tc.tile_pool(name="tap", bufs=1))
        p_c2 = ctx.enter_context(tc.tile_pool(name="c2", bufs=1))
        p_w = ctx.enter_context(tc.tile_pool(name="w", bufs=3))
        p_kc = ctx.enter_context(tc.tile_pool(name="kc", bufs=2))
        p_vs = ctx.enter_context(tc.tile_pool(name="vs", bufs=3))
        p_gb = ctx.enter_context(tc.tile_pool(name="gb", bufs=1))
        p_ob = ctx.enter_context(tc.tile_pool(name="ob", bufs=3))

        # ---- constants ----
        vtap_t = const.tile([128, NCH, KS], BF)
        nc.sync.dma_start(vtap_t[:], vtap_d[:])
        w12_t = const.tile([128, G, 8, 4], BF)
        nc.sync.dma_start(w12_t[:], w12_d[:])
        ones_t = const.tile([128, G, 4], BF)
        nc.sync.dma_start(ones_t[:], ones_d[:])
        oh_t = const.tile([4, G, 128], BF)
        nc.sync.dma_start(oh_t[:], oh_d[:].rearrange("g k m -> k g m"))
        id_t = const.tile([128, 128], BF)
        nc.sync.dma_start(id_t[:], ident_d[:])
        kb_t = const.tile([128, G, 8], F32)
        nc.sync.dma_start(kb_t[:], kb_d[:])
        vb_t = const.tile([128, 8], F32)
        nc.sync.dma_start(vb_t[:], vb_d[:])
        mask_t = const.tile([128, 2, 4, 16, 4], BF)
        nc.sync.dma_start(mask_t[:], mask_d[:])
        idx_t = const.tile([128, 2, NH, HT // 16], I16)
        nc.sync.dma_start(idx_t[:], idx_d[:])
        eps_sc_t = const.tile([4, 1], F32)
        nc.vector.memset(eps_sc_t[:], EPS_SC)
        eps_rms_t = const.tile([4, 1], F32)
        nc.vector.memset(eps_rms_t[:], EPS_RMS)

        embsT = p_emb.tile([128, 8, TOK], BF)
        xb_t = p_xn.tile([128, NCH, W], BF)
        if 2 not in phases:
            nc.vector.memset(embsT[:], 0.0)  # phase-timing builds only

        if hw_loop:
            _loop_cm = tc.For_i(0, reps, 1)
        else:
            _loop_cm = None

        for _rep in range(1 if hw_loop else reps):
            if _loop_cm is not None:
                _loop_cm.__enter__()

            # =================== phase 1 front: load + RMSNorm ===================
            if 1 in phases:
                nc.sync.dma_start(xb_t[:], hT_d[:])

                rsb = p_row.tile([4, W], F32, tag="rsb", name="rsb")
                with tc.tile_pool(name="psf", bufs=1, space="PSUM") as psf:
                    ss_cs = [psf.tile([4, 512], F32, tag=f"ssc{i}", name=f"ssc{i}")
                             for i in range(len(CHUNKS))]
                    for g in range(G):
                        for ci, (c0, cl) in enumerate(CHUNKS):
                            x2 = p_x2.tile([128, 8, 259], BF, tag="x2")
                            nc.scalar.activation(
                                x2[:, :, :cl], xb_t[:, g * 8 : (g + 1) * 8, c0 : c0 + cl],
                                AF.Square)
                            for o in range(8):
                                nc.tensor.matmul(
                                    ss_cs[ci][:, :cl], ones_t[:, g, :], x2[:, o, :cl],
                                    start=(g == 0 and o == 0), stop=(g == 3 and o == 7))
                    for ci, (c0, cl) in enumerate(CHUNKS):
                        nc.scalar.activation(rsb[:, c0 : c0 + cl], ss_cs[ci][:, :cl],
                                             AF.Sqrt, bias=eps_sc_t[:], scale=1.0 / HID)
                nc.vector.reciprocal(rsb[:], rsb[:])
                rsb_bf = p_row.tile([4, W], BF, tag="rsbbf", name="rsbbf")
                nc.scalar.activation(rsb_bf[:], rsb[:], AF.Copy)

            # ============ per-half pipeline (hoisted PSUM pools) ============
            with (
                tc.tile_pool(name="ps2", bufs=2, space="PSUM") as ps2,
                tc.tile_pool(name="ps3k", bufs=4, space="PSUM") as ps3k,
                tc.tile_pool(name="ps3r", bufs=2, space="PSUM") as ps3r,
            ):

                # ---- phase 2: gather + select into embaS. All 32 gather preps
                # issue in Pool program order with nothing between them, so the
                # DMA gather stream runs continuously. Selects (DVE) interleave
                # with the per-group rbc/xn front work below.
                embaS = p_eba.tile([128, 2, 4, 16, D], BF, tag="embaS")

                def emit_sel(hf, q):
                    ga = p_ga.tile([128, 16, 256], BF, tag="ga")
                    for jj in range(4):
                        j = q * 4 + jj
                        nc.gpsimd.dma_gather(
                            ga[:, jj * 4 : (jj + 1) * 4, :], qtab_d[j, :, :],
                            idx_t[:, hf, j, :], HT, HT, 256, queue_num=j % NQ)
                    # select: in-place mask multiply + 2 contiguous folds (DVE)
                    ga5 = ga[:].rearrange("p m (s d) -> p m s d", s=4)
                    nc.vector.tensor_tensor(
                        out=ga5, in0=ga5,
                        in1=mask_t[:, hf, q, :, :].to_broadcast((128, 16, 4, D)),
                        op=MULT)
                    nc.vector.tensor_tensor(
                        out=ga[:, :, 0:128], in0=ga[:, :, 0:128],
                        in1=ga[:, :, 128:256], op=ADD)
                    # final fold writes block-major (m' = bb*4 + j) so the
                    # transpose later reads one contiguous free dim
                    a14 = ga[:, :, 0:128].rearrange("p (j bb) h -> p j bb h", j=4)
                    nc.vector.tensor_tensor(
                        out=embaS[:, hf, q, :, :].rearrange(
                            "p (bb j) d -> p j bb d", j=4),
                        in0=a14[:, :, :, 0:D], in1=a14[:, :, :, D:128], op=ADD)

                # interleave hf0 selects with the per-group rbc/xn front
                for g in range(G):
                    if 2 in phases:
                        emit_sel(0, g)
                    if 1 in phases:
                        rbcs = p_rbc.tile([128, W], BF, tag="rbcs")
                        for c0, cl in CHUNKS:
                            rb_ps = ps3k.tile([128, HT], F32, tag="kps")
                            nc.tensor.matmul(rb_ps[:, :cl], oh_t[:, g, :],
                                             rsb_bf[:, c0 : c0 + cl], start=True, stop=True)
                            nc.scalar.activation(rbcs[:, c0 : c0 + cl], rb_ps[:, :cl], AF.Copy)
                        # in-place normalize: xb <- xb * rsqrt (per group)
                        nc.vector.tensor_tensor(
                            out=xb_t[:, g * 8 : (g + 1) * 8, :],
                            in0=xb_t[:, g * 8 : (g + 1) * 8, :],
                            in1=rbcs[:].rearrange("p (o t) -> p o t", o=1).to_broadcast((128, 8, W)),
                            op=MULT)
                if 2 in phases:
                    for q in range(4):
                        emit_sel(1, q)

                # ---- compute: per half: transposes, conv + Sq + keys, gate + value ----
                for hf in range(2):
                    pend = None
                    pg = po = 0
                    if 3 in phases:
                        # one 68-row accumulator per half (bufs=2 rotates -> no
                        # cross-half WAR stall): Sq rows 0-3, Sk rows 32-35,
                        # P rows 64-67 (engine partition bases must be 0 mod 32).
                        acc_ps = ps3r.tile([68, HT], F32, tag="acc", name=f"acc{hf}")

                        def acc_mm(g_, o_, plane, moving, first, last):
                            base = 32 * plane
                            lhsT = ones_t[:, g_, :] if plane < 2 else w12_t[:, g_, o_, :]
                            nc.tensor.matmul(acc_ps[base : base + 4, :], lhsT,
                                             moving, start=first, stop=last)
                    if 2 in phases:
                        for q in range(4):
                            embf = embaS[:, hf, q, :, :].rearrange("p m d -> p (m d)")
                            for bb in range(4):
                                for eb in range(2):
                                    pst = ps2.tile([128, 128], BF, tag="pst")
                                    c0 = (bb * 4 + 2 * eb) * D
                                    nc.tensor.transpose(pst[:], embf[:, c0 : c0 + 128], id_t[:])
                                    col = hf * HT + bb * 128
                                    nc.vector.tensor_copy(
                                        embsT[:, q * 2 + eb, col : col + 128], pst[:])
                    for g in range(G):
                        cvg = None
                        if 1 in phases:
                            # conv as PE matmuls: stationary diag(vtap[:, ci, j]),
                            # moving = normalized xn slices; accumulate 4 taps in
                            # PSUM, silu straight out of PSUM on Act.
                            dg = p_tap.tile([128, 8, KS, 128], BF, tag="dg")
                            nc.sync.dma_start(
                                dg[:], diag_d[g, :, :, :, :].rearrange("o j i q -> i o j q"))
                            cvg = p_cv.tile([128, 8, HT], BF, tag="cvg")
                            for ci8 in range(8):
                                cps = ps3k.tile([128, HT], F32, tag="kps")
                                for j in range(KS):
                                    lo = hf * HT + 3 * j
                                    nc.tensor.matmul(
                                        cps[:], dg[:, ci8, j, :],
                                        xb_t[:, g * 8 + ci8, lo : lo + HT],
                                        start=(j == 0), stop=(j == KS - 1))
                                if SIM_SILU:
                                    sg = p_kc.tile([128, HT], BF, tag="k2")
                                    nc.scalar.activation(sg[:], cps[:], AF.Sigmoid)
                                    cvf = p_kc.tile([128, HT], BF, tag="ck")
                                    nc.scalar.activation(cvf[:], cps[:], AF.Copy)
                                    nc.vector.tensor_tensor(
                                        out=cvg[:, ci8, :], in0=cvf[:], in1=sg[:], op=MULT)
                                else:
                                    nc.scalar.activation(cvg[:, ci8, :], cps[:], AF.Silu)

                        if 3 in phases and 1 in phases:
                            # conv^2 on DVE (stride-1 both operands -> 2x packing)
                            for oh2 in range(2):
                                c2 = p_c2.tile([128, 4, HT], BF, tag="c2")
                                cs4 = cvg[:, oh2 * 4 : (oh2 + 1) * 4, :]
                                nc.vector.tensor_tensor(out=c2[:], in0=cs4, in1=cs4, op=MULT)
                                for o4 in range(4):
                                    o = oh2 * 4 + o4
                                    acc_mm(g, o, 0, c2[:, o4, :],
                                           g == 0 and o == 0, g == 3 and o == 7)

                        if 3 in phases:
                            # software-pipeline the sk/pp reduction matmuls one step
                            # behind the kps chain so PE never waits on Act/DVE
                            for o in range(8):
                                kw = p_w.tile([128, 8, 128], BF, tag="kw")
                                nc.sync.dma_start(kw[:], kwT_d[g, o, :, :, :])
                                kps = ps3k.tile([128, HT], F32, tag="kps")
                                for e in range(8):
                                    nc.tensor.matmul(
                                        kps[:], kw[:, e, :],
                                        embsT[:, e, hf * HT : (hf + 1) * HT],
                                        start=(e == 0), stop=(e == 7))
                                k2 = p_kc.tile([128, HT], BF, tag="k2")
                                nc.scalar.activation(k2[:], kps[:], AF.Square,
                                                     bias=kb_t[:, g, o : o + 1])
                                ck = None
                                if 1 in phases:
                                    ck = p_kc.tile([128, HT], BF, tag="ck")
                                    nc.vector.scalar_tensor_tensor(
                                        out=ck[:], in0=kps[:], scalar=kb_t[:, g, o : o + 1],
                                        in1=cvg[:, o, :], op0=ADD, op1=MULT)
                                if pend is not None:
                                    pk2, pck = pend
                                    pf, pl = (pg == 0 and po == 0), False
                                    acc_mm(pg, po, 1, pk2[:], pf, pl)
                                    if pck is not None:
                                        acc_mm(pg, po, 2, pck[:], pf, pl)
                                pend = (k2, ck)
                                pg, po = g, o

                    # ---- gate + value for this half ----
                    if 3 in phases:
                        if pend is not None:
                            pk2, pck = pend
                            acc_mm(pg, po, 1, pk2[:], False, True)
                            if pck is not None:
                                acc_mm(pg, po, 2, pck[:], False, True)
                        ra = p_row.tile([4, HT], F32, tag="grow", name=f"ra{hf}")
                        nc.scalar.activation(ra[:], acc_ps[0:4, :], AF.Sqrt,
                                             bias=eps_rms_t[:], scale=1.0 / HID)
                        rb = p_row.tile([4, HT], F32, tag="grow", name=f"rb{hf}")
                        nc.scalar.activation(rb[:], acc_ps[32:36, :], AF.Sqrt,
                                             bias=eps_rms_t[:], scale=1.0 / HID)
                        rm = p_row.tile([4, HT], F32, tag="grow", name=f"rm{hf}")
                        nc.vector.tensor_tensor(out=rm[:], in0=ra[:], in1=rb[:], op=MULT)
                        ri = p_row.tile([4, HT], F32, tag="grow", name=f"ri{hf}")
                        nc.vector.reciprocal(ri[:], rm[:])
                        gp = p_row.tile([4, HT], F32, tag="grow", name=f"gp{hf}")
                        nc.vector.scalar_tensor_tensor(
                            out=gp[:], in0=acc_ps[64:68, :], scalar=1.0 / 32.0,
                            in1=ri[:], op0=MULT, op1=MULT)
                        gate_bf = p_row.tile([4, HT], BF, tag="gbf", name=f"gbf{hf}")
                        nc.scalar.activation(gate_bf[:], gp[:], AF.Sigmoid)

                        def value_chain(dt):
                            vw = p_w.tile([128, 8, 128], BF, tag="vw")
                            nc.sync.dma_start(vw[:], vwT_d[dt, :, :, :])
                            vps = ps3k.tile([128, HT], F32, tag="kps")
                            for e in range(8):
                                nc.tensor.matmul(
                                    vps[:], vw[:, e, :],
                                    embsT[:, e, hf * HT : (hf + 1) * HT],
                                    start=(e == 0), stop=(e == 7))
                            vsb = p_vs.tile([128, HT], BF, tag="vsb")
                            nc.scalar.activation(vsb[:], vps[:], AF.Identity,
                                                 bias=vb_t[:, dt : dt + 1])
                            return vsb

                        # prefetch two gate-independent value chains so the PE
                        # has work while the gate rows (Act/DVE + table
                        # reloads) compute; the gbc broadcast follows
                        pre = [value_chain(0), value_chain(1), value_chain(2)]
                        gbc = p_gb.tile([128, G, HT], BF, tag="gbc")
                        for g in range(G):
                            gb_ps = ps3k.tile([128, HT], F32, tag="kps")
                            nc.tensor.matmul(gb_ps[:], oh_t[:, g, :], gate_bf[:],
                                             start=True, stop=True)
                            nc.scalar.activation(gbc[:, g, :], gb_ps[:], AF.Copy)

                        for dt in range(8):
                            vsb = pre[dt] if dt < 3 else value_chain(dt)
                            for g in range(G):
                                ob = p_ob.tile([128, HT], BF, tag="ob")
                                nc.vector.tensor_tensor(
                                    out=ob[:], in0=vsb[:], in1=gbc[:, g, :], op=MULT)
                                nc.sync.dma_start(
                                    outT_d[g, dt, :, hf * HT : (hf + 1) * HT], ob[:])

            if _loop_cm is not None:
                _loop_cm.__exit__(None, None, None)

    nc.finalize()
    return nc


def _host_prep(hidden_states, hash_ids, emb_table, conv_w, sc_norm_w,
               value_w, value_b, key_w, key_b, norm1_w, norm2_w):
    # quad table [NH, NUNITS, 256] bf16
    qt = np.zeros((NH, NUNITS * 4, D), dtype=BF16)
    for j in range(NH):
        n = LIST_OF_N[j]
        qt[j, :n] = np.asarray(emb_table[OFFSETS[j] : OFFSETS[j] + n], dtype=np.float32).astype(BF16)
    qtab = np.ascontiguousarray(qt.reshape(NH, NUNITS, 256))

    vw = np.asarray(value_w, np.float32).astype(BF16)
    vwT = np.empty((8, 128, 8, 128), dtype=BF16)
    for dtile in range(8):
        blk = vw[dtile * 128 : (dtile + 1) * 128, :]
        vwT[dtile] = blk.T.reshape(8, 128, 128).transpose(1, 0, 2)
    kw = np.asarray(key_w, np.float32).astype(BF16)
    kwT = np.empty((G, 8, 128, 8, 128), dtype=BF16)
    for g in range(G):
        for ot in range(8):
            blk = kw[g, ot * 128 : (ot + 1) * 128, :]
            kwT[g, ot] = blk.T.reshape(8, 128, 128).transpose(1, 0, 2)

    cw = np.asarray(conv_w, np.float32)
    vtap_full = cw[:, 0, :] * np.asarray(sc_norm_w, np.float32).reshape(C)[:, None]  # [C, KS]
    vtap = np.ascontiguousarray(vtap_full.reshape(NCH, 128, KS).transpose(1, 0, 2).astype(BF16))
    diag = np.zeros((G, 8, KS, 128, 128), dtype=BF16)
    rr = np.arange(128)
    for g in range(G):
        for o in range(8):
            for j in range(KS):
                diag[g, o, j, rr, rr] = vtap_full[(g * 8 + o) * 128 + rr, j].astype(BF16)

    w12_full = (np.asarray(norm1_w, np.float32) * np.asarray(norm2_w, np.float32)).reshape(C)
    w12p = np.zeros((128, G, 8, 4), dtype=BF16)
    for g in range(G):
        for o in range(8):
            w12p[:, g, o, g] = w12_full[g * HID + o * 128 : g * HID + (o + 1) * 128].astype(BF16)
    onesp = np.zeros((128, G, 4), dtype=BF16)
    for g in range(G):
        onesp[:, g, g] = 1.0
    oh = np.zeros((G, 4, 128), dtype=BF16)
    for g in range(G):
        oh[g, g, :] = 1.0
    ident = np.eye(128, dtype=BF16)
    kb = np.ascontiguousarray(np.asarray(key_b, np.float32).reshape(G, 8, 128).transpose(2, 0, 1))
    vb = np.ascontiguousarray(np.asarray(value_b, np.float32).reshape(8, 128).T)

    shared = dict(qtab=qtab, vwT=vwT, kwT=kwT, vtap=vtap, diag=diag, w12pad=w12p,
                  onespad=onesp, onehot=oh, ident=ident, kb=kb, vb=vb)

    hs = np.asarray(hidden_states, np.float32).reshape(B, T, C)
    hid = np.asarray(hash_ids, np.int64)
    in_maps = []
    for core in range(N_CORES):
        b, h = core // 2, core % 2
        t0 = h * TOK
        xpad = np.zeros((W, C), dtype=np.float32)
        lo = max(0, t0 - HALO)
        xpad[HALO - (t0 - lo) :] = hs[b, lo : t0 + TOK]
        # [128, NCH, W]: hT[p, ci, t] = xpad[t, ci*128 + p]
        hT = np.ascontiguousarray(
            xpad.reshape(W, NCH, 128).transpose(2, 1, 0).astype(BF16))

        hashes = hid[b, t0 : t0 + TOK, :]                     # [TOK, NH]
        units = (hashes // 4).astype(np.int16)
        sel = (hashes % 4).astype(np.int64)
        idx16 = np.zeros((128, 2, NH, HT // 16), dtype=np.int16)
        for hf in range(2):
            for j in range(NH):
                u = units[hf * HT : (hf + 1) * HT, j]          # [HT]
                wrapped = u.reshape(HT // 16, 16).T            # [16, HT//16]
                for grp in range(8):
                    idx16[grp * 16 : (grp + 1) * 16, hf, j, :] = wrapped
        # mask[p, hf, q, jj*4+blk, s] = (sel[hf*512 + blk*128 + p, q*4+jj] == s)
        mask = np.zeros((128, 2, 4, 4, 4, 4), dtype=BF16)
        selr = sel.reshape(2, 4, 128, NH)                      # [hf, blk, p, j]
        for hf in range(2):
            for q in range(4):
                for jj in range(4):
                    for blk in range(4):
                        s = selr[hf, blk, :, q * 4 + jj]       # [128]
                        mask[np.arange(128), hf, q, jj, blk, s] = 1.0
        mask = np.ascontiguousarray(mask.reshape(128, 2, 4, 16, 4))

        in_maps.append(dict(shared, hT=hT, idx16=idx16, mask=mask))
    return in_maps


def kernel(**inputs):
    if "nc" not in _CACHE:
        _CACHE["nc"] = _build_bass()
    nc = _CACHE["nc"]
    in_maps = _host_prep(**inputs)
    res = run_bass_kernel_spmd(nc, in_maps, core_ids=list(range(N_CORES)))
    out = np.empty((B, T, G, HID), dtype=np.float32)
    for core in range(N_CORES):
        b, h = core // 2, core % 2
        t0 = h * TOK
        oT = res.results[core]["outT"]  # [G, 8, 128, TOK] bf16
        out[b, t0 : t0 + TOK] = (
            oT.astype(np.float32).reshape(G, HID, TOK).transpose(2, 0, 1))
    return out

